# revision 69
# baseline (speedup 1.0000x reference)
"""DFFN Trainium2 kernel: proj_in 1x1 -> 8x8-patch rfft2*filt*irfft2 ->
gated GELU -> 1x1 -> depthwise 3x3 -> 1x1 -> +residual.

Data-parallel over batch: 8 images, one per NeuronCore.  ~176us/core
(TimelineSim; down from 193us, 887us original).  Act/DVE are the binding
engines (~94-98% busy in steady state), PE ~90%, Pool/DMA below.

Key ideas:
  - bf16 I/O: x is cast to bf16 on the host (and pre-arranged per band
    in patch-major order, so proj_in's stationary operands are contiguous
    128-column slices) and the output DMAs back as bf16.  Halves HBM
    traffic; the branch is small and out = x + branch tolerates ~4e-3.
  - proj_in runs flipped (x 2-patch chunk stationary, w_in^T moving), so
    its output lands with patch pixels on partitions - the layout the
    per-channel FFT-filter maps M_c (blockdiag(Mc^T, Mc^T) matmuls)
    contract over.
  - The FFT-filter weights ship as the bare 64x64 McT blocks (1MB) and
    the 4MB blockdiag m2 operand is built on-device during pipeline fill
    (gpsimd memsets + DVE 4x copies, emitted per B-group in the order
    B(0) consumes them).  This unblocks B(0) ~8us earlier than DMAing
    the full m2, and the fill DMA queue is need-ordered: x(0) halves,
    mct, x(1) halves, c10.
  - The entire tail (w_before -> depthwise 3x3 -> w_out) is fused into
    C_k = W_o diag(w_dw[:,k]) W_b per tap, applied as fp8e4m3 DoubleRow
    matmuls (two taps per instruction at 0.5 cycles/row) whose moving
    operands are shifted reads of a halo'd fp8 g-slab; they accumulate
    straight into the 128 output channels in PSUM.  A DG_SCALE=64
    pre-scale keeps C_k out of the fp8 subnormal range and is divided
    back out by the PSUM eviction.
  - The +x residual is an SBUF-only tensor_add placed mostly on the
    otherwise-idle GPSIMD engine (which may not touch PSUM); for the
    last 3 bands it moves to DVE so the drain isn't gated by gpsimd's
    slow (0.42-efficiency) adds.
  - Output DMAs go out per 4-row quarter as soon as that quarter's
    evict+residual land, instead of one band-sized DMA at the end.
  - The whole thing is software-pipelined: step s emits A(s), B(s-1),
    T(s-2), tail(s-4); the last two tail steps double up (their halo/
    seam producers are already emitted).  PSUM: proj_in gets its own
    double-buffered pool; the other stages rotate through a shared
    6-bank pool.  outb is triple-buffered.

Walrus constraints found the hard way: matmul stationary APs allow only
one free dimension (moving APs are flexible), GPSIMD cannot access PSUM,
and InstTensorScalarPtr APs are limited to partition + 2 free dims.

Hardware-vs-TimelineSim lessons (this session): several emission-order /
engine-assignment changes that TimelineSim accepts crash the real device
(NRT INTERNAL/UNRECOVERABLE): interleaving A/B group emission, and
moving one steady-state residual add from gpsimd to DVE.  Every change
here was re-validated on the 8-core axon HW path.  A crashed run can
wedge the device; the next run usually recovers it - retry before
concluding a config is bad.
"""

import sys

sys.path.insert(0, "/opt/trn_rl_repo")

import numpy as np
import ml_dtypes
from contextlib import ExitStack

import concourse.bass as bass
import concourse.mybir as mybir
import concourse.tile as tile
from concourse.bass_utils import run_bass_kernel_spmd
from concourse.masks import make_identity

F32 = mybir.dt.float32
BF16 = mybir.dt.bfloat16
FP8 = mybir.dt.float8e4
BF = ml_dtypes.bfloat16
E4M3 = ml_dtypes.float8_e4m3fn
DG_SCALE = 64.0

B, C, H, W = 8, 128, 256, 256
HALF = C // 2
P = 8
BAND = 16            # image rows per band
N_CORES = 8


# --------------------------------------------------------------------------
# host-side weight preprocessing
# --------------------------------------------------------------------------

def _prep_weights(fft_filt, w_in, w_before, w_dw, w_out):
    # M_c: per-channel 64x64 map patch -> irfft2(rfft2(patch) * filt_c).
    E = np.eye(P * P, dtype=np.float64).reshape(P * P, P, P)
    FB = np.fft.rfft2(E)                                    # [64, 8, 5]
    prod = FB[None] * fft_filt.astype(np.float64)[:, None]  # [C, 64, 8, 5]
    cols = np.fft.irfft2(prod, s=(P, P)).reshape(C, P * P, P * P)
    # cols[c, k, :] is column k of M_c, i.e. cols[c] = M_c^T = the lhsT we
    # need (lhsT[k_in, m_out] = M_c[m_out, k_in]).
    McT = cols  # [C, 64in, 64out]
    # mct layout: [64 part, C*64]; the kernel builds the block-diag
    # m2 = blockdiag(McT, McT) per channel on-device (memset + 4x copies)
    mct = np.ascontiguousarray(McT.transpose(1, 0, 2).reshape(64, C * 64))

    winT = np.ascontiguousarray(w_in.T)                     # [c_in, c_out]

    # c10[:, k*128:(k+1)*128] = C_k^T = W_b^T diag(w_dw[:,k]) W_o^T
    # (whole tail w_before -> dw tap k -> w_out as one 64->128 matrix),
    # duplicated on both partition halves so either slab half-slice can be
    # the matmul contraction.  fp8e4m3 scaled by DG_SCALE (values ~1e-3
    # would be subnormal unscaled); the psO eviction divides it back out.
    # Slot 8 = ZERO block (DoubleRow pairs with tap 8 in slot 9).
    wdw9 = w_dw.reshape(HALF, 9).astype(np.float64)
    wbT = w_before.T.astype(np.float64)                     # [cc_in, c_out]
    woT = w_out.T.astype(np.float64)                        # [cc, 128]
    c10 = np.zeros((128, 10 * 128), dtype=np.float64)
    for k in range(9):
        s9 = k if k < 8 else 9
        blk = (wbT * wdw9[None, :, k]) @ woT                # [64, 128]
        c10[:64, s9 * 128:(s9 + 1) * 128] = blk
        c10[64:, s9 * 128:(s9 + 1) * 128] = blk

    return (
        mct.astype(BF),
        winT.astype(BF),
        (c10 * DG_SCALE).astype(E4M3),
    )


# --------------------------------------------------------------------------
# the tile kernel (per core, one image)
# --------------------------------------------------------------------------

def build_kernel(nc, n_rows=H, legalize=True,
                 act=mybir.ActivationFunctionType.Gelu, dev_rowwise=False):
    x_d = nc.dram_tensor("x", [C, n_rows, W], BF16, kind="ExternalInput").ap()
    mct_d = nc.dram_tensor("mct", [64, C * 64], BF16, kind="ExternalInput").ap()
    winT_d = nc.dram_tensor("winT", [C, C], BF16, kind="ExternalInput").ap()
    c10_d = nc.dram_tensor("c10", [128, 10 * 128], FP8, kind="ExternalInput").ap()
    out_d = nc.dram_tensor("out", [C, n_rows, W], BF16, kind="ExternalOutput").ap()

    n_bands = n_rows // BAND

    with tile.TileContext(nc) as tc, ExitStack() as ctx:
        singles = ctx.enter_context(tc.tile_pool(name="singles", bufs=1))
        xin_p = ctx.enter_context(tc.tile_pool(name="xin", bufs=7))
        abuf_p = ctx.enter_context(tc.tile_pool(name="abuf", bufs=3))
        gelu_p = ctx.enter_context(tc.tile_pool(name="gelu", bufs=2))
        g2_p = ctx.enter_context(tc.tile_pool(name="g2", bufs=3))
        slab_p = ctx.enter_context(tc.tile_pool(name="slab", bufs=4))
        outb_p = ctx.enter_context(tc.tile_pool(name="outb", bufs=3))

        ps_p = ctx.enter_context(tc.tile_pool(name="ps", bufs=6, space="PSUM"))
        psa_p = ctx.enter_context(tc.tile_pool(name="psa", bufs=2, space="PSUM"))

        # ---- load weights into SBUF once (m2 is 4MB; x-band DMAs are
        # issued first in the schedule so A(0) isn't blocked behind it) ----
        winT_sb = singles.tile([128, 128], BF16)
        nc.sync.dma_start(out=winT_sb, in_=winT_d)
        m2_sb = singles.tile([128, C * 128], BF16)
        mct_sb = singles.tile([64, C * 64], BF16)
        c10_sb = singles.tile([128, 10 * 128], FP8)
        ident = singles.tile([128, 128], BF16)
        make_identity(nc, ident)

        slabs = []      # ring of per-band g slabs (with halo)
        xbands = []     # per-band bf16 x tiles (for residual)

        abufs = []
        gelus = []
        g2s = []
        abuf_of = {}
        gelu_of = {}
        g2_of = {}
        outb_of = {}

        def do_dma(t, half=None):
            y0 = t * BAND
            if len(xbands) <= t:
                xbands.append(xin_p.tile([128, BAND * W], BF16, name="xband"))
            xband = xbands[t]
            if half is None:
                nc.sync.dma_start(out=xband, in_=x_d[:, y0:y0 + BAND, :])
            else:
                h0 = BAND // 2
                nc.sync.dma_start(
                    out=xband[:, half * h0 * W:(half + 1) * h0 * W],
                    in_=x_d[:, y0 + half * h0:y0 + (half + 1) * h0, :])

        def build_m2_group(g):
            """Build blockdiag(McT, McT) blocks of m2 for B-group g
            (channels g*16..+16 and 64+g*16..+16) from mct_sb: DVE 4x
            copies; the zero blocks are memset on gpsimd (emitted first,
            see zero_m2_group)."""
            for rng0 in (g * 16, 64 + g * 16):
                for half in (0, 1):
                    pm2 = m2_sb[half * 64:(half + 1) * 64, 0:1]
                    dst = bass.AP(
                        tensor=m2_sb.tensor,
                        offset=pm2.offset + rng0 * 128 + half * 64,
                        ap=[pm2.ap[0], [128, 16], [1, 64]],
                    )
                    src = bass.AP(
                        tensor=mct_sb.tensor,
                        offset=mct_sb.offset + rng0 * 64,
                        ap=[mct_sb.ap[0], [64, 16], [1, 64]],
                    )
                    nc.vector.tensor_copy(dst, src)

        def zero_m2_group(g):
            for rng0 in (g * 16, 64 + g * 16):
                for half in (0, 1):
                    pm2 = m2_sb[half * 64:(half + 1) * 64, 0:1]
                    dst = bass.AP(
                        tensor=m2_sb.tensor,
                        offset=pm2.offset + rng0 * 128 + (1 - half) * 64,
                        ap=[pm2.ap[0], [128, 16], [1, 64]],
                    )
                    nc.gpsimd.memset(dst, 0.0)

        def do_A_group(t, qg):
            """Stage A group: proj_in, flipped (2-patch pixels on out parts).
            lhsT for pair (h2, w2) reads xband directly: cols (pl, i, j)."""
            xband = xbands[t]
            abuf = abufs[t]
            psA = psa_p.tile([128, 512], F32, tag='psa')
            for q in range(4):
                pp = qg * 4 + q
                nc.tensor.matmul(
                    psA[:, q * 128:(q + 1) * 128],
                    xband[:, pp * 128:(pp + 1) * 128], winT_sb,
                    start=True, stop=True,
                )
            # evict 4 chunks: psA cols (q, o) -> abuf cols o*32 + pp0+q
            pp0 = qg * 4
            dst = bass.AP(
                tensor=abuf.tensor,
                offset=abuf.offset + pp0,
                ap=[abuf.ap[0], [1, 4], [32, 128]],
            )
            src = psA.rearrange("p (q o) -> p q o", q=4)
            if qg in (1, 3, 5):
                nc.vector.tensor_copy(dst, src)
            else:
                nc.scalar.copy(dst, src)

        def build_m2_group(g):
            """Build blockdiag(McT, McT) blocks of m2 for B-group g
            (channels g*16..+16 and 64+g*16..+16) from mct_sb with DVE 4x
            copies; zero blocks are memset on gpsimd (emitted first)."""
            for rng0 in (g * 16, 64 + g * 16):
                for half in (0, 1):
                    pm2 = m2_sb[half * 64:(half + 1) * 64, 0:1]
                    dst = bass.AP(
                        tensor=m2_sb.tensor,
                        offset=pm2.offset + rng0 * 128 + half * 64,
                        ap=[pm2.ap[0], [128, 16], [1, 64]],
                    )
                    src = bass.AP(
                        tensor=mct_sb.tensor,
                        offset=mct_sb.offset + rng0 * 64,
                        ap=[mct_sb.ap[0], [64, 16], [1, 64]],
                    )
                    nc.vector.tensor_copy(dst, src)

        def zero_m2_group(g):
            for rng0 in (g * 16, 64 + g * 16):
                for half in (0, 1):
                    pm2 = m2_sb[half * 64:(half + 1) * 64, 0:1]
                    dst = bass.AP(
                        tensor=m2_sb.tensor,
                        offset=pm2.offset + rng0 * 128 + (1 - half) * 64,
                        ap=[pm2.ap[0], [128, 16], [1, 64]],
                    )
                    nc.gpsimd.memset(dst, 0.0)

        def do_A(t, groups=range(8)):
            if t not in abuf_of:
                abuf_of[t] = abuf_p.tile([128, C * 32], BF16, name="abuf")
                abufs.append(abuf_of[t])
            for qg in groups:
                do_A_group(t, qg)

        def do_B(t, groups=range(4)):
            """Stage B: per-channel FFT-filter matmuls + gated GELU."""
            abuf = abufs[t]
            if t not in g2_of:
                gelu_of[t] = gelu_p.tile([128, 4 * 512], BF16, name="gelu_sb")
                g2_of[t] = g2_p.tile([128, 16 * 128], BF16, name="g2")
                gelus.append(gelu_of[t])
                g2s.append(g2_of[t])
            gelu_sb = gelu_of[t]
            g2 = g2_of[t]
            for g in groups:
                psB = ps_p.tile([128, 512], F32, tag='ps')
                for j in range(16):
                    c = g * 16 + j
                    nc.tensor.matmul(
                        psB[:, j * 32:(j + 1) * 32],
                        m2_sb[:, c * 128:(c + 1) * 128],
                        abuf[:, c * 32:(c + 1) * 32],
                        start=True, stop=True,
                    )
                nc.scalar.activation(
                    gelu_sb[:, g * 512:(g + 1) * 512], psB, act,
                )
                psB2 = ps_p.tile([128, 512], F32, tag='ps')
                for j in range(16):
                    c = 64 + g * 16 + j
                    nc.tensor.matmul(
                        psB2[:, j * 32:(j + 1) * 32],
                        m2_sb[:, c * 128:(c + 1) * 128],
                        abuf[:, c * 32:(c + 1) * 32],
                        start=True, stop=True,
                    )
                # gate into g2: col = (h2*8+w2')*128 + xh*64 + (g*16+j)
                dst = bass.AP(
                    tensor=g2.tensor,
                    offset=g2.offset + g * 16,
                    ap=[g2.ap[0], [64, 2], [1, 16], [1024, 2], [128, 8]],
                )
                src0 = bass.AP(
                    tensor=gelu_sb.tensor,
                    offset=gelu_sb.offset + g * 512,
                    ap=[gelu_sb.ap[0], [8, 2], [32, 16], [16, 2], [1, 8]],
                )
                src1 = bass.AP(
                    tensor=psB2.tensor,
                    offset=psB2.offset,
                    ap=[psB2.ap[0], [8, 2], [32, 16], [16, 2], [1, 8]],
                )
                nc.vector.tensor_mul(dst, src0, src1)

        def do_T(t):
            """Transpose to (xhalf, cc) partitions, scatter into the fp8
            halo slab (130-pitch rows, 1-px halo) in one pass."""
            g2 = g2s[t]
            slab = slab_p.tile([128, 18 * 130], FP8)
            slabs.append(slab)
            for h2 in range(2):
                psT = ps_p.tile([128, 1024], BF16, tag='ps')
                for w2p in range(8):
                    q = h2 * 8 + w2p
                    nc.tensor.transpose(
                        psT[:, w2p * 128:(w2p + 1) * 128],
                        g2[:, q * 128:(q + 1) * 128], ident)
                # psT col = w2p*128 + pl*64 + i*8 + j
                # -> slab col (1+8*h2+i)*130 + 1 + w2p*16 + pl*8 + j
                dst = bass.AP(
                    tensor=slab.tensor,
                    offset=slab.offset + (1 + 8 * h2) * 130 + 1,
                    ap=[slab.ap[0], [16, 8], [8, 2], [130, 8], [1, 8]],
                )
                src = psT.rearrange("p (w pl i j) -> p w pl i j", w=8, pl=2, i=8)
                if h2 == 1:
                    nc.vector.tensor_copy(dst, src)
                else:
                    nc.scalar.copy(dst, src)

            # zero the outer pad columns of rows 1..16 (image x=-1 / x=256)
            sl3 = slab.rearrange("p (r c) -> p r c", c=130)
            nc.gpsimd.memset(sl3[0:64, 1:17, 0:1], 0.0)
            nc.gpsimd.memset(sl3[64:128, 1:17, 129:130], 0.0)
            # seam: halo col 129 of left half <- col 1 of right half; col 0 of
            # right half <- col 128 of left half (rows 1..16)
            nc.sync.dma_start(out=sl3[0:64, 1:17, 129:130],
                              in_=sl3[64:128, 1:17, 1:2])
            nc.sync.dma_start(out=sl3[64:128, 1:17, 0:1],
                              in_=sl3[0:64, 1:17, 128:129])

            # halo rows between neighbouring bands
            if t == 0:
                nc.vector.memset(sl3[:, 0:1, :], 0.0)
            else:
                prev3 = slabs[t - 1].rearrange("p (r c) -> p r c", c=130)
                nc.gpsimd.tensor_copy(prev3[:, 17:18, :], sl3[:, 1:2, :])
                nc.gpsimd.tensor_copy(sl3[:, 0:1, :], prev3[:, 16:17, :])
            if t == n_bands - 1:
                nc.vector.memset(sl3[:, 17:18, :], 0.0)

        def do_DW(t, cis=range(4), rowwise=False):
            """Fused tail: psO = sum_k C_k g(.+delta_k) * S  +  S*x, then
            evict with a 1/S scale into bf16 outb.  fp8 DoubleRow pairs
            contract the slab half (64 g-channels) straight into the 128
            output channels; the residual rides an S-scaled identity
            matmul whose moving operand reads patch-major x."""
            slab = slabs[t]
            y0 = t * BAND
            xband = xbands[t]
            if t not in outb_of:
                outb_of[t] = outb_p.tile([128, BAND * W], BF16, name="outb")
            outb = outb_of[t]
            inv = 1.0 / DG_SCALE
            for ci in cis:
                r0 = ci * 4
                h2 = r0 // 8
                for xh in range(2):
                    psO = ps_p.tile([128, 512], F32, tag='ps')
                    pslab = slab[xh * 64:(xh + 1) * 64, 0:1]
                    pc10 = c10_sb[xh * 64:(xh + 1) * 64, 0:1]
                    for p in range(5):          # DoubleRow tap pairs
                        if p < 4:
                            ka, kb = 2 * p, 2 * p + 1
                            da = (1 + r0 + ka // 3 - 1) * 130 + 1 + ka % 3 - 1
                            db = (1 + r0 + kb // 3 - 1) * 130 + 1 + kb % 3 - 1
                        else:
                            ka = 8              # zero block pairs with tap 8
                            db = (2 + r0) * 130 + 2
                            da = db - 130
                        lhsT = bass.AP(
                            tensor=c10_sb.tensor,
                            offset=pc10.offset + ka * 128,
                            ap=[pc10.ap[0], [128, 2], [1, 128]],
                        )
                        if rowwise:
                            for r in range(4):
                                rhs = bass.AP(
                                    tensor=slab.tensor,
                                    offset=pslab.offset + da + r * 130,
                                    ap=[pslab.ap[0], [db - da, 2], [1, 128]],
                                )
                                nc.tensor.matmul(
                                    psO[:, r * 128:(r + 1) * 128], lhsT, rhs,
                                    start=(p == 0), stop=False,
                                    perf_mode=mybir.MatmulPerfMode.DoubleRow,
                                    skip_group_check=True,
                                )
                        else:
                            rhs = bass.AP(
                                tensor=slab.tensor,
                                offset=pslab.offset + da,
                                ap=[pslab.ap[0], [db - da, 2], [130, 4],
                                    [1, 128]],
                            )
                            nc.tensor.matmul(
                                psO, lhsT, rhs,
                                start=(p == 0), stop=(p == 4),
                                perf_mode=mybir.MatmulPerfMode.DoubleRow,
                                skip_group_check=True,
                            )
                    osl = bass.AP(
                        tensor=outb.tensor,
                        offset=outb.offset + r0 * W + xh * 128,
                        ap=[outb.ap[0], [W, 4], [1, 128]],
                    )
                    src = psO.rearrange("p (r x) -> p r x", r=4)
                    if xh == 0:
                        nc.vector.tensor_scalar_mul(osl, src, inv)
                    else:
                        nc.scalar.mul(osl, src, inv)
                    # residual: outb += x, all-SBUF so Pool can carry it
                    # (GPSIMD may not touch PSUM); x is patch-major.
                    osl4 = bass.AP(
                        tensor=outb.tensor,
                        offset=outb.offset + r0 * W + xh * 128,
                        ap=[outb.ap[0], [W, 4], [16, 8], [8, 2], [1, 8]],
                    )
                    xsl = bass.AP(
                        tensor=xband.tensor,
                        offset=xband.offset + (h2 * 16 + 8 * xh) * 128
                        + (r0 % 8) * 8,
                        ap=[xband.ap[0], [8, 4], [128, 8], [64, 2], [1, 8]],
                    )
                    if (ci, xh) in ((0, 0), (2, 0)) or t >= n_bands - 3:
                        nc.vector.tensor_add(osl4, osl4, xsl)
                    else:
                        nc.gpsimd.tensor_add(osl4, osl4, xsl)
                # per-quarter out DMA: rows y0+r0 .. y0+r0+4
                osl_dma = bass.AP(
                    tensor=outb.tensor,
                    offset=outb.offset + r0 * W,
                    ap=[outb.ap[0], [1, 4 * W]],
                )
                nc.sync.dma_start(out=out_d[:, y0 + r0:y0 + r0 + 4, :],
                                  in_=osl_dma)

        # software-pipelined schedule: step s runs A(s) | B(s-1) | T(s-2) |
        # tail(s-4), with x DMA prefetched 2 steps ahead.  The gap between
        # T (slab scatter + seam DMAs + halo-row copies) and the tail that
        # reads the slab hides the ~3us seam-DMA latency.
        for s in range(n_bands + 3):
            if s == 0:
                # fill-ordered DMA queue: A(0) inputs first, then the small
                # mct (m2 itself is built on-device), then x(1)
                for g in range(4):
                    zero_m2_group(g)
                do_dma(0, half=0)
                do_dma(0, half=1)
                nc.sync.dma_start(out=mct_sb, in_=mct_d)
                for g in range(4):
                    build_m2_group(g)
                do_dma(1, half=0)
                do_dma(1, half=1)
                nc.sync.dma_start(out=c10_sb, in_=c10_d)
            if s + 2 < n_bands:
                do_dma(s + 2)
            a_on = s < n_bands
            b_on = 0 <= s - 1 < n_bands
            if a_on:
                do_A(s)
            if b_on:
                do_B(s - 1)
            if 0 <= s - 2 < n_bands:
                do_T(s - 2)
            # drain doubles up: halo/seam producers for the last slabs
            # are already emitted by then
            if s <= n_bands:
                dws = [s - 4] if s >= 4 else []
            elif s == n_bands + 1:
                dws = [s - 4, s - 3]
            else:
                dws = [n_bands - 1]
            for t_dw in dws:
                do_DW(t_dw, rowwise=dev_rowwise)

    if legalize:
        _spill_matmul_waits(nc)
    return nc


def _spill_matmul_waits(nc):
    """Walrus encodes at most ONE sync-wait per compute-engine ISA
    instruction.  Tile sometimes leaves 2+ waits on one instruction; split
    the extras into standalone EventSemaphore wait instructions inserted
    just before, on the same (in-order) engine queue."""
    import concourse.mybir as mb
    skip = (mb.InstEventSemaphore,)
    n = [0]
    for f in nc.m.functions:
        for bb in f.blocks:
            out = []
            for inst in bb.instructions:
                si = inst.sync_info
                if (si is not None and len(si.on_wait) > 1
                        and not isinstance(inst, skip)
                        and getattr(inst, 'engine', None) is not None):
                    extra, keep = si.on_wait[:-1], si.on_wait[-1:]
                    for w in extra:
                        n[0] += 1
                        carrier = mb.InstEventSemaphore(
                            name=f"I-waitfix-{n[0]}", ins=[], outs=[])
                        carrier.engine = inst.engine
                        carrier.sync_info = mb.SyncInfo(
                            on_wait=[w], on_update=[])
                        out.append(carrier)
                    si.on_wait = keep
                out.append(inst)
            bb.instructions = out


# --------------------------------------------------------------------------
# public entry point
# --------------------------------------------------------------------------

_CACHE = {}


def _get_nc():
    if "nc" not in _CACHE:
        nc = bass.Bass("TRN2", target_bir_lowering=False, debug=False)
        build_kernel(nc, n_rows=H)
        _CACHE["nc"] = nc
    return _CACHE["nc"]


def _reorder_x(img, n_rows=H):
    """[C, n_rows, W] row-major -> per-band patch-major:
    col (within band t) = (h2*16 + w2)*128 + pl*64 + i*8 + j."""
    c = img.reshape(C, n_rows // BAND, 2, 8, 16, 2, 8)  # c,t,h2,i,w2,pl,j
    return np.ascontiguousarray(
        c.transpose(0, 1, 2, 4, 5, 3, 6).reshape(C, n_rows, W))


def kernel(x, fft_filt, w_in, w_before, w_dw, w_out):
    x = np.asarray(x, dtype=np.float32).astype(BF)
    mct, winT, c10 = _prep_weights(
        np.asarray(fft_filt, np.float32), np.asarray(w_in, np.float32),
        np.asarray(w_before, np.float32), np.asarray(w_dw, np.float32),
        np.asarray(w_out, np.float32))

    nc = _get_nc()
    in_maps = []
    for i in range(N_CORES):
        in_maps.append({
            "x": _reorder_x(x[i]),
            "mct": mct, "winT": winT, "c10": c10,
        })
    res = run_bass_kernel_spmd(nc, in_maps, list(range(N_CORES)))
    out = np.stack([res.results[i]["out"] for i in range(N_CORES)], axis=0)
    return out.astype(np.float32)

